# revision 11
# baseline (speedup 1.0000x reference)
"""Trainium2 Bass kernel for nn_Downsampler_80779744903457.

conv3x3(34->64, SAME) + bias + leaky_relu(0.2) + 10 iterations of
anisotropic-TV proximal-gradient smoothing + BatchNorm2d (training-mode batch
stats) - pure data parallel over the batch dim N=8 across 8 NeuronCores, with
the BN mean/var cross-core reduction done on-device via a tiny AllReduce.

Per-core layout: one batch sample.  TV state is fp16 in SBUF, with
  partition p = hq*32 + c_sub  (hq in 0..3 = 64-row H-block, c_sub = channel
  within a 32-channel group), free dims (h' in [0,H/4), w in [0,W)).
The 3x3 conv is computed as 3 PSUM-accumulated matmuls (one per kx tap) with
K = 34 channels x 3 ky taps + a ones-row that carries the conv bias.
Cross-partition stencil rows are staged through tiny SBUF->SBUF DMAs (DMA is
exempt from the engine partition-alignment rules).
"""

from contextlib import ExitStack

import numpy as np

CIN = 34
COUT = 64
TAU = 0.1
BN_EPS = 1e-5


def host_prepare(zd, yiq, conv_w, conv_b, bn_gamma, bn_beta, lmbd):
    """Host-side data prep; returns (per-core input dicts, thr)."""
    zd = np.asarray(zd)
    yiq = np.asarray(yiq)
    N, _, H, W = zd.shape
    x = np.concatenate([zd, yiq], axis=1)
    xpad = np.zeros((N, CIN + 1, H + 2, W + 2), np.float16)
    xpad[:, :CIN, 1 : H + 1, 1 : W + 1] = x.astype(np.float16)
    xpad[:, CIN] = 1.0
    wts = np.zeros((CIN * 3 + 1, 3, COUT), np.float16)
    w = np.asarray(conv_w).astype(np.float32)  # [cout, cin, ky, kx]
    for dxi in range(3):
        for dyi in range(3):
            wts[dyi * CIN : (dyi + 1) * CIN, dxi, :] = w[:, :, dyi, dxi].T.astype(
                np.float16
            )
    wts[CIN * 3, 1, :] = np.asarray(conv_b).astype(np.float16)
    bnp = np.zeros((32, 4), np.float32)
    g = np.asarray(bn_gamma).astype(np.float32)
    b = np.asarray(bn_beta).astype(np.float32)
    bnp[:, 0], bnp[:, 1] = g[0:32], b[0:32]
    bnp[:, 2], bnp[:, 3] = g[32:64], b[32:64]
    thr = float(1.0 / np.float32(lmbd))
    per_core = [
        {"xpad": np.ascontiguousarray(xpad[i]), "wts": wts, "bnp": bnp}
        for i in range(N)
    ]
    return per_core, thr


def build_tile_kernel(H=256, W=256, thr=1.0 / 30.0, n_iter=10, n_cores=8,
                      no_collective=False):
    import concourse.tile as tile
    from concourse import mybir
    from concourse._compat import with_exitstack

    F16 = mybir.dt.float16
    F32 = mybir.dt.float32
    OP = mybir.AluOpType
    AF = mybir.ActivationFunctionType

    K = CIN * 3 + 1  # 103
    H4 = H // 4
    Wp = W + 2
    n_total = n_cores * H * W

    @with_exitstack
    def kern(ctx: ExitStack, tc: tile.TileContext, outs, ins):
        nc = tc.nc
        xpad_d, wts_d, bnp_d = ins
        (y_d,) = outs

        persist = ctx.enter_context(tc.tile_pool(name="persist", bufs=1))
        statp = ctx.enter_context(tc.tile_pool(name="statp", bufs=1))

        u0 = persist.tile([128, H4, W], F16, tag="u0")
        u1 = persist.tile([128, H4, W], F16, tag="u1")
        u = [u0, u1]
        wt = persist.tile([K, 3, COUT], F16)
        bnpt = persist.tile([32, 4], F32)
        stats = statp.tile([128, 4], F32)
        nc.sync.dma_start(out=wt[:], in_=wts_d[:])
        nc.sync.dma_start(out=bnpt[:], in_=bnp_d[:])

        # ---- conv3x3: evac raw pre-activation into u tiles -----------------
        rows_per_mm = max(1, 512 // W)
        PSUM_ROWS = min(4 * rows_per_mm, H4)
        n_mm_slices = PSUM_ROWS // rows_per_mm

        with tc.tile_pool(name="convp", bufs=2) as convp, \
             tc.tile_pool(name="evacp", bufs=3) as evacp, \
             tc.tile_pool(name="cpsum", bufs=2, space="PSUM") as psum:
            for hq in range(4):
                rhs = convp.tile([K, H4, Wp], F16, tag="rhs")
                for dyi in range(3):
                    nc.sync.dma_start(
                        out=rhs[dyi * CIN : (dyi + 1) * CIN, :, :],
                        in_=xpad_d[0:CIN, hq * H4 + dyi : hq * H4 + dyi + H4, :],
                    )
                nc.sync.dma_start(
                    out=rhs[K - 1 : K, :, :], in_=xpad_d[CIN : CIN + 1, 0:H4, :]
                )
                for r0 in range(0, H4, PSUM_ROWS):
                    pt = psum.tile([COUT, PSUM_ROWS, W], F32, tag="cp")
                    for dxi in range(3):
                        for s in range(n_mm_slices):
                            rs = r0 + s * rows_per_mm
                            nc.tensor.matmul(
                                pt[:, s * rows_per_mm : (s + 1) * rows_per_mm, :],
                                wt[:, dxi, :],
                                rhs[:, rs : rs + rows_per_mm, dxi : dxi + W],
                                start=(dxi == 0),
                                stop=(dxi == 2),
                            )
                    stage = evacp.tile([COUT, PSUM_ROWS, W], F16, tag="stage")
                    nc.scalar.activation(out=stage[:], in_=pt[:], func=AF.Copy)
                    for g in range(2):
                        nc.sync.dma_start(
                            out=u[g][hq * 32 : hq * 32 + 32, r0 : r0 + PSUM_ROWS, :],
                            in_=stage[g * 32 : (g + 1) * 32, :, :],
                        )

        # ---- per group: leaky relu, xs, TV iterations ----------------------
        with tc.tile_pool(name="tvp", bufs=1) as tvp:
            xs = tvp.tile([128, H4, W], F16)
            A = tvp.tile([128, H4, W + 1], F16)
            B = tvp.tile([128, H4, W], F16)
            rowA = tvp.tile([128, W], F16)
            rowB = tvp.tile([128, W], F16)
            for g in range(2):
                ug = u[g]
                # leaky_relu(v) = v - 0.8*min(v, 0); temp reuses A's storage
                t = A[:, :, 0:W]
                nc.vector.tensor_scalar(
                    out=t[:], in0=ug[:], scalar1=0.0, scalar2=0.8,
                    op0=OP.min, op1=OP.mult,
                )
                nc.vector.tensor_tensor(out=ug[:], in0=ug[:], in1=t[:], op=OP.subtract)
                nc.vector.tensor_scalar_mul(out=xs[:], in0=ug[:], scalar1=TAU)
                nc.vector.memset(A[:], 0.0)
                nc.vector.memset(B[96:128, H4 - 1, :], 0.0)

                for _ in range(n_iter):
                    # A <- clamp(gx(u)); gx[w] at A col w+1; gx[W-1]=0 invariant
                    nc.vector.tensor_tensor(
                        out=A[:, :, 1:W],
                        in0=ug[:, :, 1:W],
                        in1=ug[:, :, 0 : W - 1],
                        op=OP.subtract,
                    )
                    nc.vector.tensor_scalar(
                        out=A[:], in0=A[:], scalar1=thr, scalar2=-thr,
                        op0=OP.min, op1=OP.max,
                    )
                    # B <- clamp(gy(u)); cross-block row via DMA staging
                    nc.vector.tensor_tensor(
                        out=B[:, 0 : H4 - 1, :],
                        in0=ug[:, 1:H4, :],
                        in1=ug[:, 0 : H4 - 1, :],
                        op=OP.subtract,
                    )
                    nc.sync.dma_start(out=rowA[0:96, :], in_=ug[32:128, 0, :])
                    nc.vector.tensor_tensor(
                        out=B[0:96, H4 - 1, :],
                        in0=rowA[0:96, :],
                        in1=ug[0:96, H4 - 1, :],
                        op=OP.subtract,
                    )
                    nc.vector.tensor_scalar(
                        out=B[:], in0=B[:], scalar1=thr, scalar2=-thr,
                        op0=OP.min, op1=OP.max,
                    )
                    # u <- (1-tau)*u + xs
                    nc.vector.scalar_tensor_tensor(
                        out=ug[:], in0=ug[:], scalar=1.0 - TAU, in1=xs[:],
                        op0=OP.mult, op1=OP.add,
                    )
                    # u += tau*cx[w] - tau*cx[w-1]
                    nc.vector.scalar_tensor_tensor(
                        out=ug[:], in0=A[:, :, 1 : W + 1], scalar=TAU, in1=ug[:],
                        op0=OP.mult, op1=OP.add,
                    )
                    nc.vector.scalar_tensor_tensor(
                        out=ug[:], in0=A[:, :, 0:W], scalar=-TAU, in1=ug[:],
                        op0=OP.mult, op1=OP.add,
                    )
                    # u += tau*cy[h] - tau*cy[h-1]
                    nc.vector.scalar_tensor_tensor(
                        out=ug[:], in0=B[:], scalar=TAU, in1=ug[:],
                        op0=OP.mult, op1=OP.add,
                    )
                    nc.vector.scalar_tensor_tensor(
                        out=ug[:, 1:H4, :], in0=B[:, 0 : H4 - 1, :], scalar=-TAU,
                        in1=ug[:, 1:H4, :], op0=OP.mult, op1=OP.add,
                    )
                    nc.sync.dma_start(out=rowB[32:128, :], in_=B[0:96, H4 - 1, :])
                    for q in range(1, 4):
                        nc.vector.scalar_tensor_tensor(
                            out=ug[32 * q : 32 * q + 32, 0, :],
                            in0=rowB[32 * q : 32 * q + 32, :],
                            scalar=-TAU,
                            in1=ug[32 * q : 32 * q + 32, 0, :],
                            op0=OP.mult, op1=OP.add,
                        )

                nc.scalar.activation(
                    out=B[:], in_=ug[:], func=AF.Copy,
                    accum_out=stats[:, 2 * g : 2 * g + 1],
                )
                nc.scalar.activation(
                    out=B[:], in_=ug[:], func=AF.Square,
                    accum_out=stats[:, 2 * g + 1 : 2 * g + 2],
                )

        # ---- BN stats: hq-reduce, AllReduce, coefficients ------------------
        sred = statp.tile([64, 4], F32)
        nc.sync.dma_start(out=sred[0:64], in_=stats[64:128])
        nc.vector.tensor_tensor(
            out=stats[0:64], in0=stats[0:64], in1=sred[0:64], op=OP.add
        )
        nc.sync.dma_start(out=sred[0:32], in_=stats[32:64])
        nc.vector.tensor_tensor(
            out=stats[0:32], in0=stats[0:32], in1=sred[0:32], op=OP.add
        )
        gst = statp.tile([32, 4], F32)
        if no_collective:
            nc.vector.tensor_copy(out=gst[:], in_=stats[0:32])
        else:
            with tc.tile_pool(name="dram", bufs=1, space="DRAM") as dramp:
                cc_in = dramp.tile([32, 4], F32)
                cc_out = dramp.tile(
                    [32, 4], F32, addr_space="Shared" if n_cores > 4 else "Local"
                )
                nc.sync.dma_start(out=cc_in[:], in_=stats[0:32])
                nc.gpsimd.collective_compute(
                    "AllReduce",
                    OP.add,
                    replica_groups=[list(range(n_cores))],
                    ins=[cc_in[:]],
                    outs=[cc_out[:]],
                )
                nc.sync.dma_start(out=gst[:], in_=cc_out[:])

        mb = statp.tile([32, 2], F32)
        vb = statp.tile([32, 2], F32)
        sc = statp.tile([128, 4], F32)
        tmp = statp.tile([32, 2], F32)
        inv_n = 1.0 / float(n_total)
        nc.vector.tensor_scalar_mul(out=mb[:], in0=gst[:, 0:4:2], scalar1=inv_n)
        nc.vector.tensor_scalar_mul(out=vb[:], in0=gst[:, 1:4:2], scalar1=inv_n)
        nc.vector.tensor_tensor(out=tmp[:], in0=mb[:], in1=mb[:], op=OP.mult)
        nc.vector.tensor_tensor(out=vb[:], in0=vb[:], in1=tmp[:], op=OP.subtract)
        # rstd = 1/sqrt(var + eps)
        epst = statp.tile([32, 1], F32)
        nc.vector.memset(epst[:], BN_EPS)
        nc.scalar.activation(out=vb[:], in_=vb[:], func=AF.Sqrt, bias=epst[:], scale=1.0)
        nc.vector.reciprocal(out=vb[:], in_=vb[:])
        nc.vector.tensor_tensor(
            out=sc[0:32, 0:4:2], in0=bnpt[:, 0:4:2], in1=vb[:], op=OP.mult
        )
        nc.vector.tensor_tensor(out=tmp[:], in0=mb[:], in1=sc[0:32, 0:4:2], op=OP.mult)
        nc.vector.tensor_tensor(
            out=sc[0:32, 1:4:2], in0=bnpt[:, 1:4:2], in1=tmp[:], op=OP.subtract
        )
        for q in range(1, 4):
            nc.sync.dma_start(out=sc[32 * q : 32 * q + 32], in_=sc[0:32])

        # ---- BN apply + output DMA ----------------------------------------
        HS = min(32, H4)
        n_s = H4 // HS
        y_r = y_d.rearrange(
            "(g c) (q s h) w -> g s q c h w", g=2, c=32, q=4, s=n_s, h=HS
        )
        with tc.tile_pool(name="outp", bufs=2) as outp:
            for g in range(2):
                for s in range(n_s):
                    ost = outp.tile([128, HS, W], F32, tag="ost")
                    nc.vector.tensor_scalar(
                        out=ost[:],
                        in0=u[g][:, s * HS : (s + 1) * HS, :],
                        scalar1=sc[:, 2 * g : 2 * g + 1],
                        scalar2=sc[:, 2 * g + 1 : 2 * g + 2],
                        op0=OP.mult,
                        op1=OP.add,
                    )
                    nc.sync.dma_start(out=y_r[g, s], in_=ost[:])

    return kern


def build_nc(H=256, W=256, thr=1.0 / 30.0, n_iter=10, n_cores=8,
             no_collective=False):
    import concourse.bacc as bacc
    import concourse.tile as tile
    from concourse import mybir

    F16 = mybir.dt.float16
    F32 = mybir.dt.float32
    K = CIN * 3 + 1

    nc = bacc.Bacc(
        "TRN2",
        target_bir_lowering=False,
        debug=False,
        enable_asserts=False,
        num_devices=n_cores,
    )
    xpad_t = nc.dram_tensor("xpad", [CIN + 1, H + 2, W + 2], F16, kind="ExternalInput")
    wts_t = nc.dram_tensor("wts", [K, 3, COUT], F16, kind="ExternalInput")
    bnp_t = nc.dram_tensor("bnp", [32, 4], F32, kind="ExternalInput")
    y_t = nc.dram_tensor("y", [COUT, H, W], F32, kind="ExternalOutput")

    kern = build_tile_kernel(H=H, W=W, thr=thr, n_iter=n_iter, n_cores=n_cores,
                             no_collective=no_collective)
    with tile.TileContext(nc) as tc:
        kern(tc, (y_t.ap(),), (xpad_t.ap(), wts_t.ap(), bnp_t.ap()))
    nc.compile()
    return nc


_NC_CACHE = {}


def kernel(zd, yiq, conv_w, conv_b, bn_gamma, bn_beta, lmbd, _trace=False):
    from concourse import bass_utils

    per_core, thr = host_prepare(zd, yiq, conv_w, conv_b, bn_gamma, bn_beta, lmbd)
    n_cores = len(per_core)
    key = (thr, n_cores)
    if key not in _NC_CACHE:
        _NC_CACHE[key] = build_nc(thr=thr, n_cores=n_cores)
    nc = _NC_CACHE[key]
    res = bass_utils.run_bass_kernel_spmd(
        nc, per_core, list(range(n_cores)), trace=_trace
    )
    out = np.stack([res.results[i]["y"] for i in range(n_cores)]).astype(np.float32)
    kernel.last_result = res
    return out
